# revision 4
# baseline (speedup 1.0000x reference)
"""CBOW negative-sampling loss kernel for Trainium2 (8 NeuronCores, SPMD).

Reference computation (all fp32):
    vo_embed  = vo @ V                        # [128]
    vi_embed  = (U.T @ vi).mean(axis=1)       # [128]
    left      = log_sigmoid(vi_embed @ vo_embed)
    neg_embed = neg_samples @ U               # [20, 128]
    right     = sum(log_sigmoid(-(neg_embed @ vi_embed)))
    out       = -(left + right)

Strategy: shard the vocab dim (100000) across 8 cores (12500 rows each).
All the heavy work is 22 GEMVs sharing one contraction over vocab:
pack [vo | neg_0..neg_19] (host, pure relayout) plus vi row-sums
(computed on device) into a 22-column stationary operand; stream
V|U row-chunks through the tensor engine accumulating into PSUM.
Each core emits a [22, 256] partial; the host sums partials over cores
(the "psum") and applies the scalar log-sigmoid epilogue.
"""

import numpy as np

import concourse.bacc as bacc
import concourse.bass as bass
import concourse.mybir as mybir
import concourse.tile as tile
from concourse.bass_utils import run_bass_kernel_spmd

# Problem shapes (hardcoded per spec nn_CBOW_55009941127479)
VOC = 100000
EMB = 128
CTX = 10
KNEG = 20
NCORES = 8
SHARD = VOC // NCORES          # 12500 vocab rows per core
KP = 125                       # contraction rows per matmul chunk (SBUF partitions)
NCHUNK = SHARD // KP           # 100 chunks per core
M = KNEG + 2                   # stationary columns: [vo, neg_0..19, vi_sum]
SLAB = 20                      # chunks per rhs DMA slab (1.28 MB per slab per matrix)
NSLAB = NCHUNK // SLAB

F32 = mybir.dt.float32


def build_nc():
    """Build the per-core Bass module (SPMD: same program on all 8 cores)."""
    nc = bacc.Bacc(
        "TRN2",
        target_bir_lowering=False,
        debug=False,
        num_devices=NCORES,
    )
    lhsT_d = nc.dram_tensor("lhsT_packed", [KP, NCHUNK * (M - 1)], F32,
                            kind="ExternalInput")
    vi_d = nc.dram_tensor("vi_t", [KP, NCHUNK * CTX], F32, kind="ExternalInput")
    V_d = nc.dram_tensor("V_s", [SHARD, EMB], F32, kind="ExternalInput")
    U_d = nc.dram_tensor("U_s", [SHARD, EMB], F32, kind="ExternalInput")
    out_d = nc.dram_tensor("partial", [M, 2 * EMB], F32, kind="ExternalOutput")

    with tile.TileContext(nc) as tc:
        with (
            tc.tile_pool(name="const", bufs=1) as cpool,
            tc.tile_pool(name="rhs", bufs=2) as rpool,
            tc.tile_pool(name="acc", bufs=1, space="PSUM") as ppool,
        ):
            lhsT_s = cpool.tile([KP, NCHUNK, M], F32)
            vi_s = cpool.tile([KP, NCHUNK, CTX], F32)

            # Small inputs: vi first (the reduce depends on it), then lhsT.
            nc.sync.dma_start(
                out=vi_s[:, :, :],
                in_=vi_d.rearrange("p (c j) -> p c j", j=CTX),
            )
            nc.sync.dma_start(
                out=lhsT_s[:, :, 0:M - 1],
                in_=lhsT_d.rearrange("p (c m) -> p c m", m=M - 1),
            )
            # vi row-sums (sum over the 10 context columns) -> lhsT column 21.
            nc.vector.reduce_sum(
                out=lhsT_s[:, :, M - 1],
                in_=vi_s[:, :, :],
                axis=mybir.AxisListType.X,
            )

            # out[m, 0:128]   = w_m @ V_chunk   (used for m=0: vo)
            # out[m, 128:256] = w_m @ U_chunk   (used for m=1..21: neg, vi)
            acc = ppool.tile([M, 2 * EMB], F32)
            V3 = V_d.rearrange("(c p) e -> p c e", p=KP)
            U3 = U_d.rearrange("(c p) e -> p c e", p=KP)
            for s in range(NSLAB):
                rhs = rpool.tile([KP, SLAB, 2, EMB], F32, tag="rhs")
                nc.sync.dma_start(out=rhs[:, :, 0, :],
                                  in_=V3[:, s * SLAB:(s + 1) * SLAB, :])
                nc.sync.dma_start(out=rhs[:, :, 1, :],
                                  in_=U3[:, s * SLAB:(s + 1) * SLAB, :])
                for j in range(SLAB):
                    c = s * SLAB + j
                    nc.tensor.matmul(
                        out=acc[:, :],
                        lhsT=lhsT_s[:, c, :],
                        rhs=rhs[:, j, :, :],
                        start=(c == 0),
                        stop=(c == NCHUNK - 1),
                    )

            out_s = cpool.tile([M, 2 * EMB], F32)
            nc.vector.tensor_copy(out_s[:, :], acc[:, :])
            nc.sync.dma_start(out=out_d[:, :], in_=out_s[:, :])
    nc.compile()
    return nc


def make_in_maps(vo, vi, neg_samples, V, U):
    """Shard + relayout the full inputs into 8 per-core input maps.

    Host work is pure data movement: slicing, stacking and axis
    permutation. No arithmetic on values happens here.
    """
    vo = np.asarray(vo, dtype=np.float32)
    vi = np.asarray(vi, dtype=np.float32)
    neg = np.asarray(neg_samples, dtype=np.float32)
    V = np.asarray(V, dtype=np.float32)
    U = np.asarray(U, dtype=np.float32)

    in_maps = []
    for r in range(NCORES):
        lo, hi = r * SHARD, (r + 1) * SHARD
        # [12500, 21] = [vo | neg.T] for this vocab shard
        W = np.concatenate([vo[lo:hi, None], neg[:, lo:hi].T], axis=1)
        lhsT_packed = np.ascontiguousarray(
            W.reshape(NCHUNK, KP, M - 1).transpose(1, 0, 2)
        ).reshape(KP, NCHUNK * (M - 1))
        vi_t = np.ascontiguousarray(
            vi[lo:hi].reshape(NCHUNK, KP, CTX).transpose(1, 0, 2)
        ).reshape(KP, NCHUNK * CTX)
        in_maps.append({
            "lhsT_packed": lhsT_packed,
            "vi_t": vi_t,
            "V_s": np.ascontiguousarray(V[lo:hi]),
            "U_s": np.ascontiguousarray(U[lo:hi]),
        })
    return in_maps


def combine_partials(partials):
    """Sum per-core partials and apply the scalar epilogue."""
    P = np.zeros((M, 2 * EMB), dtype=np.float64)
    for p in partials:
        P += p.astype(np.float64)
    vo_embed = P[0, :EMB]
    neg_embed = P[1:1 + KNEG, EMB:]
    vi_embed = P[M - 1, EMB:] / CTX

    def log_sigmoid(x):
        return -np.logaddexp(0.0, -x)

    left = log_sigmoid(vi_embed @ vo_embed)
    right = np.sum(log_sigmoid(-(neg_embed @ vi_embed)))
    return np.float32(-(left + right))


_NC = None


def kernel(vo, vi, neg_samples, V, U):
    global _NC
    if _NC is None:
        _NC = build_nc()
    in_maps = make_in_maps(vo, vi, neg_samples, V, U)
    res = run_bass_kernel_spmd(_NC, in_maps, list(range(NCORES)))
    return combine_partials([res.results[r]["partial"] for r in range(NCORES)])


# revision 7
# speedup vs baseline: 1.2525x; 1.2525x over previous
"""CBOW negative-sampling loss kernel for Trainium2 (8 NeuronCores, SPMD).

Reference computation (all fp32):
    vo_embed  = vo @ V                        # [128]
    vi_embed  = (U.T @ vi).mean(axis=1)       # [128]
    left      = log_sigmoid(vi_embed @ vo_embed)
    neg_embed = neg_samples @ U               # [20, 128]
    right     = sum(log_sigmoid(-(neg_embed @ vi_embed)))
    out       = -(left + right)

Strategy: shard the vocab dim (100000) across 8 cores (12500 rows each).
All the heavy work is 22 GEMVs sharing one contraction over vocab:
pack [vo | neg_0..neg_19] (host, pure relayout) plus vi row-sums
(computed on device) into a 22-column stationary operand; stream
V|U row-chunks through the tensor engine accumulating into PSUM.
Each core emits a [22, 256] partial; the host sums partials over cores
(the "psum") and applies the scalar log-sigmoid epilogue.
"""

import numpy as np

import concourse.bacc as bacc
import concourse.bass as bass
import concourse.mybir as mybir
import concourse.tile as tile
from concourse.bass_utils import run_bass_kernel_spmd

# Problem shapes (hardcoded per spec nn_CBOW_55009941127479)
VOC = 100000
EMB = 128
CTX = 10
KNEG = 20
NCORES = 8
SHARD = VOC // NCORES          # 12500 vocab rows per core
KP = 125                       # contraction rows per matmul chunk (SBUF partitions)
NCHUNK = SHARD // KP           # 100 chunks per core
M = KNEG + 2                   # stationary columns: [vo, neg_0..19, vi_sum]
SLAB = 20                      # chunks per rhs DMA slab (1.28 MB per slab per matrix)
NSLAB = NCHUNK // SLAB

# Vocab rows are processed in a p-major order within each slab so that every
# DMA is contiguous on both the DRAM and SBUF side:
#   shard row for (slab s, partition p, chunk-in-slab j) = s*KP*SLAB + p*SLAB + j
# The host packs lhsT/vi in the same order, so all operands agree on the
# (equivalent, order-independent) contraction over vocab.

F32 = mybir.dt.float32


def build_nc():
    """Build the per-core Bass module (SPMD: same program on all 8 cores)."""
    nc = bacc.Bacc(
        "TRN2",
        target_bir_lowering=False,
        debug=False,
        num_devices=NCORES,
    )
    lhsT_d = nc.dram_tensor("lhsT_packed", [KP, NCHUNK * M], F32,
                            kind="ExternalInput")
    vi_d = nc.dram_tensor("vi_t", [KP, NCHUNK * CTX], F32, kind="ExternalInput")
    V_d = nc.dram_tensor("V_s", [SHARD, EMB], F32, kind="ExternalInput")
    U_d = nc.dram_tensor("U_s", [SHARD, EMB], F32, kind="ExternalInput")
    out_d = nc.dram_tensor("partial", [M, 2 * EMB], F32, kind="ExternalOutput")

    with tile.TileContext(nc) as tc:
        with (
            tc.tile_pool(name="const", bufs=1) as cpool,
            tc.tile_pool(name="rhs", bufs=3) as rpool,
            tc.tile_pool(name="acc", bufs=1, space="PSUM") as ppool,
        ):
            lhsT_s = cpool.tile([KP, NCHUNK, M], F32)
            vi_s = cpool.tile([KP, NCHUNK, CTX], F32)

            # Small inputs (fully contiguous transfers). vi on the ACT HWDGE
            # ring, lhsT on the SP ring so they proceed concurrently.
            nc.scalar.dma_start(
                out=vi_s[:, :, :],
                in_=vi_d.rearrange("p (c j) -> p c j", j=CTX),
            )
            nc.sync.dma_start(
                out=lhsT_s[:, :, :],
                in_=lhsT_d.rearrange("p (c m) -> p c m", m=M),
            )
            # vi row-sums (sum over the 10 context columns) overwrite the
            # zero placeholder in lhsT column 21.
            nc.vector.reduce_sum(
                out=lhsT_s[:, :, M - 1],
                in_=vi_s[:, :, :],
                axis=mybir.AxisListType.X,
            )

            # out[m, 0:128]   = w_m @ V_chunk   (used for m=0: vo)
            # out[m, 128:256] = w_m @ U_chunk   (used for m=1..21: neg, vi)
            acc = ppool.tile([M, 2 * EMB], F32)
            # p-major row order within each slab: row = s*KP*SLAB + p*SLAB + j
            V4 = V_d.rearrange("(s p j) e -> p s j e", p=KP, j=SLAB)
            U4 = U_d.rearrange("(s p j) e -> p s j e", p=KP, j=SLAB)
            for s in range(NSLAB):
                rhs = rpool.tile([KP, 2, SLAB, EMB], F32, tag="rhs")
                # V on the SP HWDGE ring, U on the ACT ring: two DMAs in
                # flight; each is contiguous in DRAM and per-partition SBUF.
                nc.sync.dma_start(out=rhs[:, 0, :, :], in_=V4[:, s, :, :])
                nc.scalar.dma_start(out=rhs[:, 1, :, :], in_=U4[:, s, :, :])
                for j in range(SLAB):
                    c = s * SLAB + j
                    nc.tensor.matmul(
                        out=acc[:, :],
                        lhsT=lhsT_s[:, c, :],
                        rhs=rhs[:, :, j, :],
                        start=(c == 0),
                        stop=(c == NCHUNK - 1),
                    )

            out_s = cpool.tile([M, 2 * EMB], F32)
            nc.vector.tensor_copy(out_s[:, :], acc[:, :])
            nc.sync.dma_start(out=out_d[:, :], in_=out_s[:, :])
    nc.compile()
    return nc


def make_in_maps(vo, vi, neg_samples, V, U):
    """Shard + relayout the full inputs into 8 per-core input maps.

    Host work is pure data movement: slicing, stacking and axis
    permutation. No arithmetic on values happens here.
    """
    vo = np.asarray(vo, dtype=np.float32)
    vi = np.asarray(vi, dtype=np.float32)
    neg = np.asarray(neg_samples, dtype=np.float32)
    V = np.asarray(V, dtype=np.float32)
    U = np.asarray(U, dtype=np.float32)

    in_maps = []
    for r in range(NCORES):
        lo, hi = r * SHARD, (r + 1) * SHARD
        # [12500, 22] = [vo | neg.T | 0] for this vocab shard; the zero
        # column is the placeholder the device overwrites with vi row-sums.
        W = np.concatenate(
            [vo[lo:hi, None], neg[:, lo:hi].T,
             np.zeros((SHARD, 1), np.float32)], axis=1)
        # p-major slab order: row(s, p, j) = s*KP*SLAB + p*SLAB + j
        lhsT_packed = np.ascontiguousarray(
            W.reshape(NSLAB, KP, SLAB, M).transpose(1, 0, 2, 3)
        ).reshape(KP, NCHUNK * M)
        vi_t = np.ascontiguousarray(
            vi[lo:hi].reshape(NSLAB, KP, SLAB, CTX).transpose(1, 0, 2, 3)
        ).reshape(KP, NCHUNK * CTX)
        in_maps.append({
            "lhsT_packed": lhsT_packed,
            "vi_t": vi_t,
            "V_s": np.ascontiguousarray(V[lo:hi]),
            "U_s": np.ascontiguousarray(U[lo:hi]),
        })
    return in_maps


def combine_partials(partials):
    """Sum per-core partials and apply the scalar epilogue."""
    P = np.zeros((M, 2 * EMB), dtype=np.float64)
    for p in partials:
        P += p.astype(np.float64)
    vo_embed = P[0, :EMB]
    neg_embed = P[1:1 + KNEG, EMB:]
    vi_embed = P[M - 1, EMB:] / CTX

    def log_sigmoid(x):
        return -np.logaddexp(0.0, -x)

    left = log_sigmoid(vi_embed @ vo_embed)
    right = np.sum(log_sigmoid(-(neg_embed @ vi_embed)))
    return np.float32(-(left + right))


_NC = None


def kernel(vo, vi, neg_samples, V, U):
    global _NC
    if _NC is None:
        _NC = build_nc()
    in_maps = make_in_maps(vo, vi, neg_samples, V, U)
    res = run_bass_kernel_spmd(_NC, in_maps, list(range(NCORES)))
    return combine_partials([res.results[r]["partial"] for r in range(NCORES)])


# revision 11
# speedup vs baseline: 1.8336x; 1.4639x over previous
"""CBOW negative-sampling loss kernel for Trainium2 (8 NeuronCores, SPMD).

Reference computation (all fp32):
    vo_embed  = vo @ V                        # [128]
    vi_embed  = (U.T @ vi).mean(axis=1)       # [128]
    left      = log_sigmoid(vi_embed @ vo_embed)
    neg_embed = neg_samples @ U               # [20, 128]
    right     = sum(log_sigmoid(-(neg_embed @ vi_embed)))
    out       = -(left + right)

Strategy: shard the vocab dim (100000) across 8 cores (12500 rows each).
All the heavy work is 31 GEMVs sharing one contraction over vocab:
pack [vo | neg_0..neg_19 | vi_0..vi_9] (host, pure relayout) into a
31-column stationary operand; stream V|U row-chunks through the tensor
engine accumulating into PSUM.  Each core emits a [31, 256] partial; the
host sums partials over cores (the "psum"), averages the 10 vi rows, and
applies the scalar log-sigmoid epilogue.
"""

import numpy as np

import concourse.bacc as bacc
import concourse.bass as bass
import concourse.mybir as mybir
import concourse.tile as tile
from concourse.bass_utils import run_bass_kernel_spmd

# Problem shapes (hardcoded per spec nn_CBOW_55009941127479)
VOC = 100000
EMB = 128
CTX = 10
KNEG = 20
NCORES = 8
SHARD = VOC // NCORES          # 12500 vocab rows per core
KP = 125                       # contraction rows per matmul chunk (SBUF partitions)
NCHUNK = SHARD // KP           # 100 chunks per core
M = 1 + KNEG + CTX             # stationary columns: [vo, neg_0..19, vi_0..9]
SLAB = 25                      # chunks per rhs DMA slab (1.6 MB per slab per matrix)
NSLAB = NCHUNK // SLAB
USE_FP32R = True               # PE single-pass fp32r: 4x matmul throughput

# Vocab rows are processed in a p-major order within each slab so that every
# DMA is contiguous on both the DRAM and SBUF side:
#   shard row for (slab s, partition p, chunk-in-slab j) = s*KP*SLAB + p*SLAB + j
# The host packs lhsT in the same order, so all operands agree on the
# (equivalent, order-independent) contraction over vocab.

F32 = mybir.dt.float32
MM_DT = mybir.dt.float32r if USE_FP32R else F32


def build_nc():
    """Build the per-core Bass module (SPMD: same program on all 8 cores)."""
    nc = bacc.Bacc(
        "TRN2",
        target_bir_lowering=False,
        debug=False,
        num_devices=NCORES,
    )
    lhsT_d = nc.dram_tensor("lhsT_packed", [KP, NCHUNK * M], F32,
                            kind="ExternalInput")
    V_d = nc.dram_tensor("V_s", [SHARD, EMB], F32, kind="ExternalInput")
    U_d = nc.dram_tensor("U_s", [SHARD, EMB], F32, kind="ExternalInput")
    out_d = nc.dram_tensor("partial", [M, 2 * EMB], F32, kind="ExternalOutput")

    with tile.TileContext(nc) as tc:
        with (
            tc.tile_pool(name="const", bufs=1) as cpool,
            tc.tile_pool(name="rhs", bufs=3) as rpool,
            tc.tile_pool(name="acc", bufs=1, space="PSUM") as ppool,
        ):
            # All matmul operands live in SBUF as float32r (the SWDGE DMA
            # cast rounds them); the PE runs single-pass fp32r matmuls.
            lhsT_s = cpool.tile([KP, NCHUNK, M], MM_DT)

            # SWDGE (gpsimd) sprays descriptors across all 16 SDMA engines;
            # the HWDGE dynamic queues only fan out 5-wide (~130 GB/s cap).
            nc.gpsimd.dma_start(
                out=lhsT_s[:, :, :],
                in_=lhsT_d.rearrange("p (c m) -> p c m", m=M),
            )

            # out[m, 0:128]   = w_m @ V_chunk   (used for m=0: vo)
            # out[m, 128:256] = w_m @ U_chunk   (used for m=1..30: neg, vi)
            acc = ppool.tile([M, 2 * EMB], F32)
            # p-major row order within each slab: row = s*KP*SLAB + p*SLAB + j
            V4 = V_d.rearrange("(s p j) e -> p s j e", p=KP, j=SLAB)
            U4 = U_d.rearrange("(s p j) e -> p s j e", p=KP, j=SLAB)
            for s in range(NSLAB):
                rhs = rpool.tile([KP, 2, SLAB, EMB], MM_DT, tag="rhs")
                # Each transfer is contiguous in DRAM and per-partition
                # contiguous in SBUF.
                nc.gpsimd.dma_start(out=rhs[:, 0, :, :], in_=V4[:, s, :, :])
                nc.gpsimd.dma_start(out=rhs[:, 1, :, :], in_=U4[:, s, :, :])
                for j in range(SLAB):
                    c = s * SLAB + j
                    nc.tensor.matmul(
                        out=acc[:, :],
                        lhsT=lhsT_s[:, c, :],
                        rhs=rhs[:, :, j, :],
                        start=(c == 0),
                        stop=(c == NCHUNK - 1),
                    )

            out_s = cpool.tile([M, 2 * EMB], F32)
            nc.vector.tensor_copy(out_s[:, :], acc[:, :])
            nc.sync.dma_start(out=out_d[:, :], in_=out_s[:, :])
    nc.compile()
    return nc


def make_in_maps(vo, vi, neg_samples, V, U):
    """Shard + relayout the full inputs into 8 per-core input maps.

    Host work is pure data movement: slicing, stacking and axis
    permutation. No arithmetic on values happens here.
    """
    vo = np.asarray(vo, dtype=np.float32)
    vi = np.asarray(vi, dtype=np.float32)
    neg = np.asarray(neg_samples, dtype=np.float32)
    V = np.asarray(V, dtype=np.float32)
    U = np.asarray(U, dtype=np.float32)

    in_maps = []
    for r in range(NCORES):
        lo, hi = r * SHARD, (r + 1) * SHARD
        # [12500, 31] = [vo | neg.T | vi] for this vocab shard
        W = np.concatenate([vo[lo:hi, None], neg[:, lo:hi].T, vi[lo:hi]],
                           axis=1)
        # p-major slab order: row(s, p, j) = s*KP*SLAB + p*SLAB + j
        lhsT_packed = np.ascontiguousarray(
            W.reshape(NSLAB, KP, SLAB, M).transpose(1, 0, 2, 3)
        ).reshape(KP, NCHUNK * M)
        in_maps.append({
            "lhsT_packed": lhsT_packed,
            "V_s": np.ascontiguousarray(V[lo:hi]),
            "U_s": np.ascontiguousarray(U[lo:hi]),
        })
    return in_maps


def combine_partials(partials):
    """Sum per-core partials and apply the scalar epilogue."""
    P = np.zeros((M, 2 * EMB), dtype=np.float64)
    for p in partials:
        P += p.astype(np.float64)
    vo_embed = P[0, :EMB]
    neg_embed = P[1:1 + KNEG, EMB:]
    vi_embed = P[1 + KNEG:, EMB:].sum(axis=0) / CTX

    def log_sigmoid(x):
        return -np.logaddexp(0.0, -x)

    left = log_sigmoid(vi_embed @ vo_embed)
    right = np.sum(log_sigmoid(-(neg_embed @ vi_embed)))
    return np.float32(-(left + right))


_NC = None


def kernel(vo, vi, neg_samples, V, U):
    global _NC
    if _NC is None:
        _NC = build_nc()
    in_maps = make_in_maps(vo, vi, neg_samples, V, U)
    res = run_bass_kernel_spmd(_NC, in_maps, list(range(NCORES)))
    return combine_partials([res.results[r]["partial"] for r in range(NCORES)])
